# revision 1
# baseline (speedup 1.0000x reference)
"""Raw-bacc (no Tile) BoundaryLoss kernel — explicit semaphores.

Per core: sm/dm DRAM [128, 12288] f32 (batches {2k,2k+1}, classes 1:4).
All data SBUF-resident; the two input tensors stream on the two HWDGE
rings (SP carries sm, ACT carries dm) in uneven chunks — small first so
the vector engine starts early, small last so the tail is short.
DVE: per-chunk mul + reduce into acc columns; final column reduce.
PE: single ones-vector matmul partition reduction into PSUM.

The Bass construction-time preamble (const-AP memsets + all-engine
barrier, ~3.5 us of event-semaphore latency) is stripped from the BIR —
nothing in this kernel uses the const APs, and `ones` is memset by the
vector engine instead. Semaphores start at zero (NRT zeroes them at
model load and in its end-of-execution postamble), so no explicit
cleanup tail is required for re-execution.
"""

import numpy as np

import concourse.bass as bass
from concourse import bacc, mybir
from concourse.bass_utils import run_bass_kernel_spmd

N_CORES = 8
P = 128
N, C, H, W = 16, 4, 512, 512
CLS = C - 1
PER_CORE_N = N // N_CORES
FREE = PER_CORE_N * CLS * H * W // P  # 12288

# per-tensor chunk sizes (free elems); 1 col = 1 KiB of DMA across both tensors
CHUNKS = [512, 1024, 1536, 2048, 2048, 2048, 2048, 512, 512]
assert sum(CHUNKS) == FREE
NT = len(CHUNKS)
OFFS = [sum(CHUNKS[:t]) for t in range(NT)]
MAXC = max(CHUNKS)

_nc_cache = None


def build_nc():
    global _nc_cache
    if _nc_cache is not None:
        return _nc_cache

    nc = bacc.Bacc(None, target_bir_lowering=False)
    # Bass.__init__ emitted const-AP memsets + a full event-sem barrier
    # (~3.5 us of event-semaphore latency before any DMA can issue);
    # nothing in this kernel reads the const APs, so strip the memsets
    # and the barrier. Register init (TPBBaseLd/RegisterMove) and the
    # module-entry call stay.
    preamble = [
        i
        for i in nc.main_func.blocks[0].instructions
        if type(i).__name__ in ("InstMemset", "InstDrain", "InstEventSemaphore")
    ]

    f32 = mybir.dt.float32
    sm = nc.dram_tensor("sm", [P, FREE], f32, kind="ExternalInput")
    dm = nc.dram_tensor("dm", [P, FREE], f32, kind="ExternalInput")
    out = nc.dram_tensor("out", [1, 1], f32, kind="ExternalOutput")

    bufA = nc.alloc_sbuf_tensor("bufA", [P, FREE], f32).ap()
    bufB = nc.alloc_sbuf_tensor("bufB", [P, FREE], f32).ap()
    prod = nc.alloc_sbuf_tensor("prod", [P, 2 * MAXC], f32).ap()
    acc = nc.alloc_sbuf_tensor("acc", [P, NT], f32).ap()
    acc1 = nc.alloc_sbuf_tensor("acc1", [P, 1], f32).ap()
    ones = nc.alloc_sbuf_tensor("ones", [P, 1], f32).ap()
    res = nc.alloc_sbuf_tensor("res", [1, 1], f32).ap()
    psum = nc.alloc_psum_tensor("psum", [1, 1], f32).ap()

    # SWDGE third-row experiment regressed (steals ramp bandwidth from
    # the pacing-critical HWDGE rings) — keep everything on the 2 rings.
    SWDGE_CHUNKS = ()

    # The SP ring measures ~10% slower than the ACT ring, so its tensor
    # finishes last. Rebalance: sm's LAST chunk rides the ACT ring as its
    # final transfer — ring finish times even out and no mid-stream pair
    # is delayed (a mid-stream split measurably stalls the in-order DVE).
    SPLIT_T = NT - 1

    s_sm = [nc.alloc_semaphore(f"s_sm{t}") for t in range(NT)]
    s_smb = nc.alloc_semaphore("s_smb")
    s_dm = [nc.alloc_semaphore(f"s_dm{t}") for t in range(NT)]
    s_dve = nc.alloc_semaphore("s_dve")
    s_ones = nc.alloc_semaphore("s_ones")
    s_acc = nc.alloc_semaphore("s_acc")
    s_mm = nc.alloc_semaphore("s_mm")
    s_res = nc.alloc_semaphore("s_res")
    s_out = nc.alloc_semaphore("s_out")

    def chunk(ap, t):
        return ap[:, OFFS[t] : OFFS[t] + CHUNKS[t]]

    with nc.Block() as block:

        @block.sync
        def _(sync):
            for t in range(NT):
                if t in SWDGE_CHUNKS:
                    continue
                if t != SPLIT_T:
                    sync.dma_start(chunk(bufA, t), chunk(sm, t)).then_inc(s_sm[t], 16)
            sync.wait_ge(s_res, 1)
            sync.dma_start(out[:], res[:]).then_inc(s_out, 16)

        @block.scalar
        def _(scalar):
            for t in range(NT):
                if t in SWDGE_CHUNKS:
                    continue
                scalar.dma_start(chunk(bufB, t), chunk(dm, t)).then_inc(s_dm[t], 16)
                if t == SPLIT_T:
                    scalar.dma_start(chunk(bufA, t), chunk(sm, t)).then_inc(s_smb, 16)

        @block.gpsimd
        def _(gpsimd):
            for t in SWDGE_CHUNKS:
                gpsimd.dma_start(chunk(bufA, t), chunk(sm, t)).then_inc(s_sm[t], 16)
                gpsimd.dma_start(chunk(bufB, t), chunk(dm, t)).then_inc(s_dm[t], 16)

        @block.vector
        def _(vector):
            vector.memset(ones[:], 1.0).then_inc(s_ones, 1)
            for t in range(NT):
                if t >= 2:
                    # prod[t%2] free again (reduce_{t-2} done) — WAR guard
                    vector.wait_ge(s_dve, 2 * (t - 2) + 2)
                if t == SPLIT_T:
                    vector.wait_ge(s_smb, 16)
                else:
                    vector.wait_ge(s_sm[t], 16)
                pr = prod[:, bass.ts(t % 2, MAXC)][:, : CHUNKS[t]]
                i = vector.tensor_mul(pr, chunk(bufA, t), chunk(bufB, t))
                i._wait_ge(s_dm[t], 16)
                i.then_inc(s_dve, 1)
                i = vector.reduce_sum(
                    acc[:, t : t + 1], pr, axis=mybir.AxisListType.X
                )
                i._wait_ge(s_dve, 2 * t + 1)
                i.then_inc(s_dve, 1)
            vector.wait_ge(s_dve, 2 * NT)
            i = vector.reduce_sum(acc1[:], acc[:], axis=mybir.AxisListType.X)
            i.then_inc(s_acc, 1)
            vector.wait_ge(s_mm, 1)
            vector.tensor_copy(res[:], psum[:]).then_inc(s_res, 1)

        @block.tensor
        def _(tensor):
            tensor.wait_ge(s_ones, 1)
            tensor.wait_ge(s_acc, 1)
            nc.tensor.matmul(psum[:], acc1[:], ones[:], start=True, stop=True).then_inc(
                s_mm, 1
            )

    # strip the construction-time preamble
    bb0 = nc.main_func.blocks[0]
    for inst in preamble:
        bb0.instructions.remove(inst)

    nc.compile()
    _nc_cache = nc
    return nc


def make_in_maps(softmax_output, distance_maps):
    sm = np.ascontiguousarray(softmax_output[:, 1:, :, :]).reshape(N, CLS * H * W)
    dm = np.ascontiguousarray(distance_maps[:, 1:, :, :]).reshape(N, CLS * H * W)
    in_maps = []
    for k in range(N_CORES):
        rows = slice(k * PER_CORE_N, (k + 1) * PER_CORE_N)
        in_maps.append(
            {
                "sm": sm[rows].reshape(P, FREE),
                "dm": dm[rows].reshape(P, FREE),
            }
        )
    return in_maps


def run(softmax_output, distance_maps, **spmd_kwargs):
    nc = build_nc()
    in_maps = make_in_maps(softmax_output, distance_maps)
    r = run_bass_kernel_spmd(nc, in_maps, core_ids=list(range(N_CORES)), **spmd_kwargs)
    total = sum(float(res_["out"][0, 0]) for res_ in r.results)
    loss = np.float32(total / (N * CLS))
    return np.asarray(loss, dtype=np.float32), r


def kernel(softmax_output, target, distance_maps):
    softmax_output = np.asarray(softmax_output, dtype=np.float32)
    distance_maps = np.asarray(distance_maps, dtype=np.float32)
    loss, _ = run(softmax_output, distance_maps)
    return loss



# revision 7
# speedup vs baseline: 1.3196x; 1.3196x over previous
"""Raw-bacc (no Tile) BoundaryLoss kernel — explicit semaphores.

Per core: sm/dm DRAM [128, 12288] f32 (batches {2k,2k+1}, classes 1:4).
All data SBUF-resident; the two input tensors stream on the two HWDGE
rings (SP carries sm, ACT carries dm). Chunks are large first (DMA
efficiency) and taper to 64 cols at the end so the DVE tail after the
last byte lands is tiny.

DVE: one fused scalar_tensor_tensor (InstTensorScalarPtr) per chunk —
out=(sm*1.0)*dm with accum_out = free-dim sum, i.e. product + reduce in
a single DVE pass (half the DVE work of mul+reduce; InstTensorTensorReduce
compiles but wedges the device on this image) writing one accumulator
column per chunk. The [128, NT] accumulator is DMA'd out
directly; the host sums the 8*128*NT partials (removes the PE
partition-reduce matmul + copy + 3 semaphore hops from the tail).

The Bass construction-time preamble (const-AP memsets + all-engine
barrier, ~3.5 us of event-semaphore latency) is stripped from the BIR —
nothing in this kernel uses the const APs. Semaphores start at zero
(NRT zeroes them at model load and in its end-of-execution postamble),
so no explicit cleanup tail is required for re-execution.
"""

import numpy as np

import concourse.bass as bass
from concourse import bacc, mybir
from concourse.bass_utils import run_bass_kernel_spmd

N_CORES = 8
P = 128
N, C, H, W = 16, 4, 512, 512
CLS = C - 1
PER_CORE_N = N // N_CORES
FREE = PER_CORE_N * CLS * H * W // P  # 12288

# per-tensor chunk sizes (free elems); 1 col = 1 KiB of DMA across both
# tensors. Large first (DMA efficiency), tapering tail so the last
# chunk's fused DVE op is ~0.2 us.
CHUNKS = [2048, 2048, 2048, 2048, 2048, 1024, 512, 320, 128, 64]
assert sum(CHUNKS) == FREE
NT = len(CHUNKS)
OFFS = [sum(CHUNKS[:t]) for t in range(NT)]
MAXC = max(CHUNKS)

_nc_cache = None


def build_nc():
    global _nc_cache
    if _nc_cache is not None:
        return _nc_cache

    nc = bacc.Bacc(None, target_bir_lowering=False)
    # Bass.__init__ emitted const-AP memsets + a full event-sem barrier
    # (~3.5 us of event-semaphore latency before any DMA can issue);
    # nothing in this kernel reads the const APs, so strip the memsets
    # and the barrier. Register init (TPBBaseLd/RegisterMove) and the
    # module-entry call stay.
    preamble = [
        i
        for i in nc.main_func.blocks[0].instructions
        if type(i).__name__ in ("InstMemset", "InstDrain", "InstEventSemaphore")
    ]

    f32 = mybir.dt.float32
    sm = nc.dram_tensor("sm", [P, FREE], f32, kind="ExternalInput")
    dm = nc.dram_tensor("dm", [P, FREE], f32, kind="ExternalInput")
    out = nc.dram_tensor("out", [P, NT], f32, kind="ExternalOutput")

    bufA = nc.alloc_sbuf_tensor("bufA", [P, FREE], f32).ap()
    bufB = nc.alloc_sbuf_tensor("bufB", [P, FREE], f32).ap()
    # write-only product sink for the fused op (never read). Full-FREE
    # layout: each chunk writes a disjoint region (the race detector
    # rejects even the benign same-engine WAW of a shared sink).
    prod = nc.alloc_sbuf_tensor("prod", [P, FREE], f32).ap()
    acc = nc.alloc_sbuf_tensor("acc", [P, NT], f32).ap()

    # The SP ring measures ~10% slower than the ACT ring, so its tensor
    # finishes last. Rebalance: sm's LAST chunk rides the ACT ring as its
    # final transfer — ring finish times even out and no mid-stream pair
    # is delayed (a mid-stream split measurably stalls the in-order DVE).
    SPLIT_T = NT - 1

    s_sm = [nc.alloc_semaphore(f"s_sm{t}") for t in range(NT)]
    s_smb = nc.alloc_semaphore("s_smb")
    s_dm = [nc.alloc_semaphore(f"s_dm{t}") for t in range(NT)]
    s_acc = nc.alloc_semaphore("s_acc")
    s_out = nc.alloc_semaphore("s_out")

    def chunk(ap, t):
        return ap[:, OFFS[t] : OFFS[t] + CHUNKS[t]]

    with nc.Block() as block:

        @block.sync
        def _(sync):
            for t in range(NT):
                if t != SPLIT_T:
                    sync.dma_start(chunk(bufA, t), chunk(sm, t)).then_inc(s_sm[t], 16)
            sync.wait_ge(s_acc, 1)
            sync.dma_start(out[:], acc[:]).then_inc(s_out, 16)

        @block.scalar
        def _(scalar):
            for t in range(NT):
                scalar.dma_start(chunk(bufB, t), chunk(dm, t)).then_inc(s_dm[t], 16)
                if t == SPLIT_T:
                    scalar.dma_start(chunk(bufA, t), chunk(sm, t)).then_inc(s_smb, 16)

        @block.vector
        def _(vector):
            for t in range(NT):
                if t == SPLIT_T:
                    vector.wait_ge(s_smb, 16)
                else:
                    vector.wait_ge(s_sm[t], 16)
                i = vector.scalar_tensor_tensor(
                    out=chunk(prod, t),
                    in0=chunk(bufA, t),
                    scalar=1.0,
                    in1=chunk(bufB, t),
                    op0=mybir.AluOpType.mult,
                    op1=mybir.AluOpType.mult,
                    accum_out=acc[:, t : t + 1],
                )
                i._wait_ge(s_dm[t], 16)
                if t == NT - 1:
                    i.then_inc(s_acc, 1)

    # strip the construction-time preamble
    bb0 = nc.main_func.blocks[0]
    for inst in preamble:
        bb0.instructions.remove(inst)

    nc.compile()
    _nc_cache = nc
    return nc


def make_in_maps(softmax_output, distance_maps):
    sm = np.ascontiguousarray(softmax_output[:, 1:, :, :]).reshape(N, CLS * H * W)
    dm = np.ascontiguousarray(distance_maps[:, 1:, :, :]).reshape(N, CLS * H * W)
    in_maps = []
    for k in range(N_CORES):
        rows = slice(k * PER_CORE_N, (k + 1) * PER_CORE_N)
        in_maps.append(
            {
                "sm": sm[rows].reshape(P, FREE),
                "dm": dm[rows].reshape(P, FREE),
            }
        )
    return in_maps


def run(softmax_output, distance_maps, **spmd_kwargs):
    nc = build_nc()
    in_maps = make_in_maps(softmax_output, distance_maps)
    r = run_bass_kernel_spmd(nc, in_maps, core_ids=list(range(N_CORES)), **spmd_kwargs)
    total = sum(float(res_["out"].astype(np.float64).sum()) for res_ in r.results)
    loss = np.float32(total / (N * CLS))
    return np.asarray(loss, dtype=np.float32), r


def kernel(softmax_output, target, distance_maps):
    softmax_output = np.asarray(softmax_output, dtype=np.float32)
    distance_maps = np.asarray(distance_maps, dtype=np.float32)
    loss, _ = run(softmax_output, distance_maps)
    return loss


# revision 8
# speedup vs baseline: 1.7986x; 1.3630x over previous
"""Raw-bacc (no Tile) BoundaryLoss kernel — explicit semaphores.

Per core: sm/dm DRAM [128, 12288] f32 (batches {2k,2k+1}, classes 1:4).
All data SBUF-resident; the two input tensors stream on the two HWDGE
rings (SP carries sm, ACT carries dm). Chunks are large first (DMA
efficiency) and taper to 64 cols at the end so the DVE tail after the
last byte lands is tiny.

DVE: one fused scalar_tensor_tensor (InstTensorScalarPtr) per chunk —
out=(sm*1.0)*dm with accum_out = free-dim sum, i.e. product + reduce in
a single DVE pass (half the DVE work of mul+reduce; InstTensorTensorReduce
compiles but wedges the device on this image) writing one accumulator
column per chunk. The [128, NT] accumulator is DMA'd out
directly; the host sums the 8*128*NT partials (removes the PE
partition-reduce matmul + copy + 3 semaphore hops from the tail).

The Bass construction-time preamble (const-AP memsets + all-engine
barrier, ~3.5 us of event-semaphore latency) is stripped from the BIR —
nothing in this kernel uses the const APs. Semaphores start at zero
(NRT zeroes them at model load and in its end-of-execution postamble),
so no explicit cleanup tail is required for re-execution.
"""

import numpy as np

import concourse.bass as bass
from concourse import bacc, mybir
from concourse.bass_utils import run_bass_kernel_spmd

N_CORES = 8
P = 128
N, C, H, W = 16, 4, 512, 512
CLS = C - 1
PER_CORE_N = N // N_CORES
FREE = PER_CORE_N * CLS * H * W // P  # 12288

# per-tensor chunk sizes (free elems); 1 col = 1 KiB of DMA across both
# tensors. Large first (DMA efficiency), tapering tail so the last
# chunk's fused DVE op is ~0.2 us.
CHUNKS = [2048, 2048, 2048, 2048, 2048, 1024, 512, 320, 128, 64]
assert sum(CHUNKS) == FREE
NT = len(CHUNKS)
OFFS = [sum(CHUNKS[:t]) for t in range(NT)]
MAXC = max(CHUNKS)

_nc_cache = None


def build_nc():
    global _nc_cache
    if _nc_cache is not None:
        return _nc_cache

    nc = bacc.Bacc(None, target_bir_lowering=False)
    # Bass.__init__ emitted const-AP memsets + a full event-sem barrier
    # (~3.5 us of event-semaphore latency before any DMA can issue);
    # nothing in this kernel reads the const APs, so strip the memsets
    # and the barrier. Register init (TPBBaseLd/RegisterMove) and the
    # module-entry call stay.
    preamble = [
        i
        for i in nc.main_func.blocks[0].instructions
        if type(i).__name__ in ("InstMemset", "InstDrain", "InstEventSemaphore")
    ]

    f32 = mybir.dt.float32
    bf16 = mybir.dt.bfloat16
    sm = nc.dram_tensor("sm", [P, FREE], bf16, kind="ExternalInput")
    dm = nc.dram_tensor("dm", [P, FREE], bf16, kind="ExternalInput")
    out = nc.dram_tensor("out", [P, NT], f32, kind="ExternalOutput")

    bufA = nc.alloc_sbuf_tensor("bufA", [P, FREE], bf16).ap()
    bufB = nc.alloc_sbuf_tensor("bufB", [P, FREE], bf16).ap()
    # write-only product sink for the fused op (never read). Full-FREE
    # layout: each chunk writes a disjoint region (the race detector
    # rejects even the benign same-engine WAW of a shared sink).
    prod = nc.alloc_sbuf_tensor("prod", [P, FREE], bf16).ap()
    acc = nc.alloc_sbuf_tensor("acc", [P, NT], f32).ap()

    # The SP ring measures ~10% slower than the ACT ring, so its tensor
    # finishes last. Rebalance: sm's LAST chunk rides the ACT ring as its
    # final transfer — ring finish times even out and no mid-stream pair
    # is delayed (a mid-stream split measurably stalls the in-order DVE).
    SPLIT_T = NT - 1

    s_sm = [nc.alloc_semaphore(f"s_sm{t}") for t in range(NT)]
    s_smb = nc.alloc_semaphore("s_smb")
    s_dm = [nc.alloc_semaphore(f"s_dm{t}") for t in range(NT)]
    s_acc = nc.alloc_semaphore("s_acc")
    s_out = nc.alloc_semaphore("s_out")

    def chunk(ap, t):
        return ap[:, OFFS[t] : OFFS[t] + CHUNKS[t]]

    with nc.Block() as block:

        @block.sync
        def _(sync):
            for t in range(NT):
                if t != SPLIT_T:
                    sync.dma_start(chunk(bufA, t), chunk(sm, t)).then_inc(s_sm[t], 16)
            sync.wait_ge(s_acc, 1)
            sync.dma_start(out[:], acc[:]).then_inc(s_out, 16)

        @block.scalar
        def _(scalar):
            for t in range(NT):
                scalar.dma_start(chunk(bufB, t), chunk(dm, t)).then_inc(s_dm[t], 16)
                if t == SPLIT_T:
                    scalar.dma_start(chunk(bufA, t), chunk(sm, t)).then_inc(s_smb, 16)

        @block.vector
        def _(vector):
            for t in range(NT):
                if t == SPLIT_T:
                    vector.wait_ge(s_smb, 16)
                else:
                    vector.wait_ge(s_sm[t], 16)
                i = vector.scalar_tensor_tensor(
                    out=chunk(prod, t),
                    in0=chunk(bufA, t),
                    scalar=1.0,
                    in1=chunk(bufB, t),
                    op0=mybir.AluOpType.mult,
                    op1=mybir.AluOpType.mult,
                    accum_out=acc[:, t : t + 1],
                )
                i._wait_ge(s_dm[t], 16)
                if t == NT - 1:
                    i.then_inc(s_acc, 1)

    # strip the construction-time preamble
    bb0 = nc.main_func.blocks[0]
    for inst in preamble:
        bb0.instructions.remove(inst)

    nc.compile()
    _nc_cache = nc
    return nc


def make_in_maps(softmax_output, distance_maps):
    import ml_dtypes

    # bf16 device representation: halves HBM traffic; with f32
    # accumulation the loss rel-err is ~2e-4, far inside the 2e-2 gate.
    sm = softmax_output[:, 1:, :, :].astype(ml_dtypes.bfloat16).reshape(N, CLS * H * W)
    dm = distance_maps[:, 1:, :, :].astype(ml_dtypes.bfloat16).reshape(N, CLS * H * W)
    in_maps = []
    for k in range(N_CORES):
        rows = slice(k * PER_CORE_N, (k + 1) * PER_CORE_N)
        in_maps.append(
            {
                "sm": sm[rows].reshape(P, FREE),
                "dm": dm[rows].reshape(P, FREE),
            }
        )
    return in_maps


def run(softmax_output, distance_maps, **spmd_kwargs):
    nc = build_nc()
    in_maps = make_in_maps(softmax_output, distance_maps)
    r = run_bass_kernel_spmd(nc, in_maps, core_ids=list(range(N_CORES)), **spmd_kwargs)
    total = sum(float(res_["out"].astype(np.float64).sum()) for res_ in r.results)
    loss = np.float32(total / (N * CLS))
    return np.asarray(loss, dtype=np.float32), r


def kernel(softmax_output, target, distance_maps):
    softmax_output = np.asarray(softmax_output, dtype=np.float32)
    distance_maps = np.asarray(distance_maps, dtype=np.float32)
    loss, _ = run(softmax_output, distance_maps)
    return loss
